# revision 26
# baseline (speedup 1.0000x reference)
"""Trainium2 Bass kernel for a GRU decoder with attention + vocab classifier.

Model (per reference):
  h0 = ihs @ W_hm.T + b_hm ; ctx0 = 0
  per step t (teacher forcing):
    gi = [x_t, ctx] @ W_ih.T + b_ih ; gh = h @ W_hh.T + b_hh
    r = sig(gi_r + gh_r); z = sig(gi_z + gh_z); n = tanh(gi_n + r * gh_n)
    h' = (1-z)*n + z*h
    scores = einsum('bsh,bh->bs', enc, h'); attn = softmax(scores)
    ctx' = einsum('bsh,bs->bh', enc, attn)
    logits_t = [ctx', h'] @ W_cls.T + b_cls

Distribution (8 cores):
  - recurrence batch-sharded: core i owns batches [4i, 4i+4); exact fp32
  - pred = [ctx', h'] states AllGathered in chunks of 8 timesteps
  - classifier vocab-sharded (fp32r matmuls): core i owns vocab rows
    [4000i, 4000(i+1)), computes its slice for ALL batches; host concats.

Numerics: the recurrence uses exact-fp32 matmuls (fp32r ~ tf32 is too
coarse for this chaotic RNN's feedback path). Sigmoids are computed as
sig(x) = (tanh(x/2)+1)/2 so every ACT function (tanh/exp/copy) lives in
one activation table -> no per-step ACT_TABLE_LOADs. The 1/2 factors are
folded: tanh gets scale=0.5; W_hh/b_hh n-slices are pre-halved on host;
the z-path applies one extra 0.5 multiply.

Batch-partition constraint: compute-engine SBUF accesses must start at
partition 0/32/64/96, so cross-matmul outputs place batch b at partition
32b ("spread" layout) via small selector matrices, and packed forms are
derived with strided-free copies.
"""

import numpy as np

import concourse.bacc as bacc
import concourse.bass as bass
import concourse.mybir as mybir
from concourse import masks, tile
from concourse.bass_utils import run_bass_kernel_spmd

B, SRC, T, H, E, V = 32, 128, 64, 256, 256, 32000
NCORES = 8
BL = B // NCORES          # 4 local batches
VS = V // NCORES          # 4000 vocab shard
CHUNK_T = 8               # timesteps per AllGather/classifier chunk
NVC = 500                 # vocab cols per classifier matmul
GX_AHEAD = 6              # per-step Gx row DMA prefetch distance

dt = mybir.dt
F32, F32R, I32 = dt.float32, dt.float32r, dt.int32
Alu = mybir.AluOpType
Act = mybir.ActivationFunctionType


def build_nc(t_steps=T):
    nc = bacc.Bacc(None, target_bir_lowering=False, debug=True)
    n_tok = BL * t_steps                      # local tokens, t-major: col = 4t+b
    n_mt = (n_tok + 127) // 128               # token tiles for Gx

    # ---------------- DRAM I/O ----------------
    encT_d = nc.dram_tensor("encT", [2, 128, BL * SRC], F32, kind="ExternalInput")
    encR_d = nc.dram_tensor("encR", [SRC, BL * H], F32, kind="ExternalInput")
    wicT_d = nc.dram_tensor("wicT", [2, 128, 3 * H], F32, kind="ExternalInput")
    whhT_d = nc.dram_tensor("whhT", [2, 128, 3 * H], F32, kind="ExternalInput")
    wixT_d = nc.dram_tensor("wixT", [2, 128, 3 * H], F32, kind="ExternalInput")
    whmT_d = nc.dram_tensor("whmT", [2, 128, H], F32, kind="ExternalInput")
    ihsT_d = nc.dram_tensor("ihsT", [2, 128, BL], F32, kind="ExternalInput")
    bgx_d = nc.dram_tensor("bgx", [1, 3 * H], F32, kind="ExternalInput")
    bhhn_d = nc.dram_tensor("bhhn", [1, H], F32, kind="ExternalInput")
    bhm_d = nc.dram_tensor("bhm", [1, H], F32, kind="ExternalInput")
    bcls_d = nc.dram_tensor("bcls", [1, VS], F32, kind="ExternalInput")
    emb_d = nc.dram_tensor("emb", [V, E], F32, kind="ExternalInput")
    tok_d = nc.dram_tensor("tok", [n_mt, 128, 1], I32, kind="ExternalInput")
    wclsT_d = nc.dram_tensor("wclsT", [4, 128, VS], F32R, kind="ExternalInput")
    # spread/gather selector matrices (constants; batch b <-> partition 32b)
    spreadp_d = nc.dram_tensor("spreadp", [BL, 97], F32, kind="ExternalInput")
    s97_d = nc.dram_tensor("s97", [97, 97], F32, kind="ExternalInput")
    gatherp_d = nc.dram_tensor("gatherp", [97, BL], F32, kind="ExternalInput")
    # rows ordered [chunk][core][t_local][b_local]; host permutes to [B, T, VS]
    out_d = nc.dram_tensor("logits", [NCORES * n_tok, VS], F32, kind="ExternalOutput")

    with tile.TileContext(nc) as tc:
        with (
            tc.tile_pool(name="const", bufs=1) as const,
            tc.tile_pool(name="state", bufs=2) as state,
            tc.tile_pool(name="gxs", bufs=GX_AHEAD + 2) as gxsp,
            tc.tile_pool(name="gath", bufs=2) as gath,
            tc.tile_pool(name="clsst", bufs=2) as clsst,
            tc.tile_pool(name="ps_g", bufs=1, space="PSUM") as ps_g,
            tc.tile_pool(name="ps_sc", bufs=1, space="PSUM") as ps_sc,
            tc.tile_pool(name="ps_t", bufs=1, space="PSUM") as ps_t,
            tc.tile_pool(name="ps_cls", bufs=1, space="PSUM") as ps_cls,
            tc.tile_pool(name="dram", bufs=2, space="DRAM") as dram,
        ):
            # ------------- load constants -------------
            ident = const.tile([128, 128], F32)
            masks.make_identity(nc, ident[:])

            encT_sb = [const.tile([128, BL * SRC], F32, tag=f"encT{k}", name=f"encT{k}") for k in range(2)]
            for k in range(2):
                nc.sync.dma_start(out=encT_sb[k][:], in_=encT_d[k])
            encR_sb = const.tile([SRC, BL * H], F32)
            nc.sync.dma_start(out=encR_sb[:], in_=encR_d[:])

            wic_sb = [const.tile([128, 3 * H], F32, tag=f"wic{k}", name=f"wic{k}") for k in range(2)]
            whh_sb = [const.tile([128, 3 * H], F32, tag=f"whh{k}", name=f"whh{k}") for k in range(2)]
            wix_sb = [const.tile([128, 3 * H], F32, tag=f"wix{k}", name=f"wix{k}") for k in range(2)]
            whm_sb = [const.tile([128, H], F32, tag=f"whm{k}", name=f"whm{k}") for k in range(2)]
            ihsT_sb = [const.tile([128, BL], F32, tag=f"ihsT{k}", name=f"ihsT{k}") for k in range(2)]
            for k in range(2):
                nc.sync.dma_start(out=wic_sb[k][:], in_=wicT_d[k])
                nc.sync.dma_start(out=whh_sb[k][:], in_=whhT_d[k])
                nc.sync.dma_start(out=wix_sb[k][:], in_=wixT_d[k])
                nc.sync.dma_start(out=whm_sb[k][:], in_=whmT_d[k])
                nc.sync.dma_start(out=ihsT_sb[k][:], in_=ihsT_d[k])

            wcls_sb = [const.tile([128, VS], F32R, tag=f"wcls{k}", name=f"wcls{k}") for k in range(4)]
            for k in range(4):
                nc.sync.dma_start(out=wcls_sb[k][:], in_=wclsT_d[k])

            bgx_rep = const.tile([128, 3 * H], F32)
            nc.sync.dma_start(out=bgx_rep[:], in_=bgx_d[:].to_broadcast([128, 3 * H]))
            bhhn_rep = const.tile([BL, H], F32)
            nc.sync.dma_start(out=bhhn_rep[:], in_=bhhn_d[:].to_broadcast([BL, H]))
            bhm_rep = const.tile([BL, H], F32)
            nc.sync.dma_start(out=bhm_rep[:], in_=bhm_d[:].to_broadcast([BL, H]))
            bcls_rep = const.tile([128, VS], F32)
            nc.sync.dma_start(out=bcls_rep[:], in_=bcls_d[:].to_broadcast([128, VS]))
            spreadp = const.tile([BL, 97], F32)
            nc.sync.dma_start(out=spreadp[:], in_=spreadp_d[:])
            s97 = const.tile([97, 97], F32)
            nc.sync.dma_start(out=s97[:], in_=s97_d[:])
            gatherp = const.tile([97, BL], F32)
            nc.sync.dma_start(out=gatherp[:], in_=gatherp_d[:])

            # ------------- embedding gather + Gx -------------
            gx_dram = dram.tile([n_mt * 128, 3 * H], F32, bufs=1)
            X = [const.tile([128, E], F32, tag=f"X{m}", name=f"X{m}") for m in range(n_mt)]
            XT = [const.tile([128, n_mt * 128], F32, tag=f"XT{k}", name=f"XT{k}") for k in range(2)]
            for m in range(n_mt):
                idx = const.tile([128, 1], I32, tag=f"idx{m}")
                nc.sync.dma_start(out=idx[:], in_=tok_d[m])
                nc.gpsimd.indirect_dma_start(
                    out=X[m][:],
                    out_offset=None,
                    in_=emb_d[:],
                    in_offset=bass.IndirectOffsetOnAxis(ap=idx[:, :1], axis=0),
                )
                tokf = const.tile([128, 1], F32, tag=f"tokf{m}")
                nc.vector.tensor_copy(out=tokf[:], in_=idx[:])
                nc.vector.tensor_scalar_min(tokf[:], tokf[:], 1.0)
                # zero out padding_idx=0 rows
                nc.vector.tensor_scalar_mul(X[m][:], X[m][:], tokf[:, 0:1])
                # transpose X -> XT
                pxt = ps_cls.tile([128, 768], F32, tag="cls")
                for k in range(2):
                    nc.tensor.transpose(
                        out=pxt[:, k * 128:(k + 1) * 128],
                        in_=X[m][:, k * 128:(k + 1) * 128],
                        identity=ident[:],
                    )
                for k in range(2):
                    nc.vector.tensor_copy(
                        out=XT[k][:, m * 128:(m + 1) * 128],
                        in_=pxt[:, k * 128:(k + 1) * 128],
                    )
            for m in range(n_mt):
                pgx = ps_cls.tile([128, 768], F32, tag="cls")
                for lo, hi in ((0, 512), (512, 768)):
                    for k in range(2):
                        nc.tensor.matmul(
                            out=pgx[:, lo:hi],
                            lhsT=XT[k][:, m * 128:(m + 1) * 128],
                            rhs=wix_sb[k][:, lo:hi],
                            start=(k == 0),
                            stop=(k == 1),
                        )
                gx_sb = state.tile([128, 3 * H], F32, tag="gx_sb")
                nc.vector.tensor_tensor(out=gx_sb[:], in0=pgx[:], in1=bgx_rep[:], op=Alu.add)
                nc.sync.dma_start(out=gx_dram[m * 128:(m + 1) * 128, :], in_=gx_sb[:])

            # per-step Gx row staging (DMA prefetch; arbitrary partitions OK)
            gx_t = {}

            def prefetch_gx(t):
                if t < t_steps and t not in gx_t:
                    g = gxsp.tile([BL, 3 * H], F32, tag="gxt", name=f"gxt{t}")
                    nc.sync.dma_start(out=g[:], in_=gx_dram[BL * t:BL * (t + 1), :])
                    gx_t[t] = g

            for t0 in range(GX_AHEAD):
                prefetch_gx(t0)

            # ------------- h0 -------------
            ph0 = ps_g.tile([BL, 2 * H], F32, tag="rz")
            for k in range(2):
                nc.tensor.matmul(
                    out=ph0[:, 0:H],
                    lhsT=ihsT_sb[k][:],
                    rhs=whm_sb[k][:],
                    start=(k == 0),
                    stop=(k == 1),
                )
            h_prev = state.tile([BL, H], F32, tag="h")
            nc.vector.tensor_tensor(out=h_prev[:], in0=ph0[:, 0:H], in1=bhm_rep[:], op=Alu.add)
            h0T = const.tile([128, 2 * BL], F32)
            pt0 = ps_t.tile([128, 2 * 128], F32, tag="pt")
            for k in range(2):
                nc.tensor.transpose(
                    out=pt0[:, k * BL:(k + 1) * BL],
                    in_=h_prev[:, k * 128:(k + 1) * 128],
                    identity=ident[0:BL, 0:BL],
                )
            nc.vector.tensor_copy(out=h0T[:], in_=pt0[:, 0:2 * BL])

            # predT: [ctxT(2 tiles); hT(2 tiles)], col 4t+b holds step-t output state
            predT = [const.tile([128, n_tok], F32, tag=f"predT{k}", name=f"predT{k}") for k in range(4)]

            # spread-layout scratch: batch b lives at partition 32*b
            sc_spread = const.tile([97, SRC], F32)
            nc.vector.memset(sc_spread[:], 0.0)
            ctx_spread = const.tile([97, H], F32)
            nc.vector.memset(ctx_spread[:], 0.0)

            # classifier work queue, flushed gradually
            cls_queue = []

            def emit_cls_unit(gp_tiles, c, mt, n_m_cols, n):
                pcls = ps_cls.tile([128, 768], F32, tag="cls")
                for k in range(4):
                    nc.tensor.matmul(
                        out=pcls[:n_m_cols, 0:NVC],
                        lhsT=gp_tiles[k][:, mt * 128: mt * 128 + n_m_cols],
                        rhs=wcls_sb[k][:, n * NVC:(n + 1) * NVC],
                        start=(k == 0),
                        stop=(k == 3),
                    )
                st = clsst.tile([128, NVC], F32, tag="clsst")
                nc.vector.tensor_tensor(
                    out=st[:n_m_cols, :],
                    in0=pcls[:n_m_cols, 0:NVC],
                    in1=bcls_rep[:n_m_cols, n * NVC:(n + 1) * NVC],
                    op=Alu.add,
                )
                r0 = NCORES * BL * CHUNK_T * c + mt * 128
                ap = out_d[r0:r0 + n_m_cols, n * NVC:(n + 1) * NVC]
                nc.sync.dma_start(out=ap, in_=st[:n_m_cols, :])

            def flush_cls(k_units):
                for _ in range(k_units):
                    if cls_queue:
                        cls_queue.pop(0)()

            # ------------- recurrence -------------
            for t in range(t_steps):
                prefetch_gx(t + GX_AHEAD)
                gx = gx_t[t]

                def ctxT_ap(k, t=t):
                    return predT[k][:, BL * (t - 1):BL * t]

                def hT_ap(k, t=t):
                    if t == 0:
                        return h0T[:, k * BL:(k + 1) * BL]
                    return predT[2 + k][:, BL * (t - 1):BL * t]

                # gate matmuls
                pz = ps_g.tile([BL, 2 * H], F32, tag="rz")
                pin = ps_g.tile([BL, H], F32, tag="in")
                phn = ps_g.tile([BL, H], F32, tag="hn")
                if t > 0:
                    for k in range(2):
                        nc.tensor.matmul(
                            out=pz[:], lhsT=ctxT_ap(k), rhs=wic_sb[k][:, 0:512],
                            start=(k == 0), stop=False,
                        )
                    for k in range(2):
                        nc.tensor.matmul(
                            out=pin[:], lhsT=ctxT_ap(k), rhs=wic_sb[k][:, 512:768],
                            start=(k == 0), stop=(k == 1),
                        )
                for k in range(2):
                    nc.tensor.matmul(
                        out=pz[:], lhsT=hT_ap(k), rhs=whh_sb[k][:, 0:512],
                        start=(t == 0 and k == 0), stop=(k == 1),
                    )
                for k in range(2):
                    # n-slice of whh is pre-halved on host
                    nc.tensor.matmul(
                        out=phn[:], lhsT=hT_ap(k), rhs=whh_sb[k][:, 512:768],
                        start=(k == 0), stop=(k == 1),
                    )

                # gate math; sig(x) = (tanh(x/2)+1)/2
                a_rz = state.tile([BL, 2 * H], F32, tag="a_rz")
                nc.vector.tensor_tensor(out=a_rz[:], in0=pz[:], in1=gx[:, 0:512], op=Alu.add)
                u_rz = state.tile([BL, 2 * H], F32, tag="u_rz")
                nc.scalar.activation(u_rz[:], a_rz[:], Act.Tanh, scale=0.5)
                # hnb = 0.5*(gh_n + b_hh_n)   (0.5 pre-folded into whh/bhhn)
                hnb = state.tile([BL, H], F32, tag="hnb")
                nc.vector.tensor_tensor(out=hnb[:], in0=phn[:], in1=bhhn_rep[:], op=Alu.add)
                # r*gh_n = (u_r+1)*hnb = u_r*hnb + hnb
                m1 = state.tile([BL, H], F32, tag="m1")
                nc.vector.tensor_tensor(out=m1[:], in0=u_rz[:, 0:H], in1=hnb[:], op=Alu.mult)
                s1 = state.tile([BL, H], F32, tag="s1")
                nc.vector.tensor_tensor(out=s1[:], in0=m1[:], in1=hnb[:], op=Alu.add)
                inn = state.tile([BL, H], F32, tag="inn")
                if t > 0:
                    nc.vector.tensor_tensor(out=inn[:], in0=pin[:], in1=gx[:, 512:768], op=Alu.add)
                    inn_ap = inn[:]
                else:
                    inn_ap = gx[:, 512:768]
                npre = state.tile([BL, H], F32, tag="npre")
                nc.vector.tensor_tensor(out=npre[:], in0=s1[:], in1=inn_ap, op=Alu.add)
                nn = state.tile([BL, H], F32, tag="nn")
                nc.scalar.activation(nn[:], npre[:], Act.Tanh)
                # h' = nn + z*(h-nn), z = (u_z+1)/2  -> h' = nn + 0.5*(u_z*d + d)
                d = state.tile([BL, H], F32, tag="d")
                nc.vector.tensor_tensor(out=d[:], in0=h_prev[:], in1=nn[:], op=Alu.subtract)
                m2 = state.tile([BL, H], F32, tag="m2")
                nc.vector.tensor_tensor(out=m2[:], in0=u_rz[:, H:2 * H], in1=d[:], op=Alu.mult)
                s2 = state.tile([BL, H], F32, tag="s2")
                nc.vector.tensor_tensor(out=s2[:], in0=m2[:], in1=d[:], op=Alu.add)
                s2h = state.tile([BL, H], F32, tag="s2h")
                nc.vector.tensor_scalar_mul(s2h[:], s2[:], 0.5)
                h_new = state.tile([BL, H], F32, tag="h")
                nc.vector.tensor_tensor(out=h_new[:], in0=nn[:], in1=s2h[:], op=Alu.add)
                h_prev = h_new

                # h_new -> spread-transposed [128, 97] (col 32b = batch b)
                # and packed predT h-half, via selector matmuls
                pt = ps_t.tile([128, 2 * 128], F32, tag="pt")
                hts = [state.tile([128, 97], F32, tag=f"hts{k}", name=f"hts{k}")
                       for k in range(2)]
                for k in range(2):
                    nc.tensor.matmul(
                        out=pt[:, 128 * k:128 * k + 97],
                        lhsT=h_new[:, k * 128:(k + 1) * 128],
                        rhs=spreadp[:],
                        start=True, stop=True,
                    )
                for k in range(2):
                    eng = nc.vector.tensor_copy if k == 0 else nc.scalar.copy
                    eng(out=hts[k][:], in_=pt[:, 128 * k:128 * k + 97])
                for k in range(2):
                    eng = nc.scalar.copy if k == 0 else nc.vector.tensor_copy
                    eng(out=predT[2 + k][:, BL * t:BL * (t + 1)],
                        in_=pt[:, 128 * k:128 * (k + 1)]
                        .rearrange("p (b u) -> p b u", u=32)[:, :, 0:1])

                # scores: psum rows 32b hold batch-b scores
                psc = ps_sc.tile([97, BL * H], F32, tag="sc")
                for k in range(2):
                    nc.tensor.matmul(
                        out=psc[:, 0:BL * SRC],
                        lhsT=hts[k][:],
                        rhs=encT_sb[k][:],
                        start=(k == 0), stop=(k == 1),
                    )
                # extract diag rows into spread layout (partition 32*b)
                for b in range(BL):
                    eng = nc.vector.tensor_copy if b % 2 == 0 else nc.scalar.copy
                    eng(out=sc_spread[32 * b:32 * b + 1, :],
                        in_=psc[32 * b:32 * b + 1, b * SRC:(b + 1) * SRC])

                # softmax (unnormalized; 1/sum folded into ctx extraction)
                negmax = state.tile([97, 1], F32, tag="negmax")
                nc.vector.tensor_reduce(
                    out=negmax[:], in_=sc_spread[:], axis=mybir.AxisListType.X,
                    op=Alu.max, negate=True,
                )
                attn = state.tile([97, SRC], F32, tag="attn")
                sume = state.tile([97, 1], F32, tag="sume")
                nc.scalar.activation(
                    attn[:], sc_spread[:], Act.Exp,
                    bias=negmax[:, 0:1], accum_out=sume[:, 0:1],
                )
                rcp = state.tile([97, 1], F32, tag="rcp")
                nc.vector.reciprocal(rcp[:], sume[:])

                # attn [97, 128] -> spread-transposed atT [128, 97] via s97
                pat = ps_t.tile([128, 2 * 128], F32, tag="pt")
                nc.tensor.matmul(out=pat[:, 0:97], lhsT=attn[:], rhs=s97[:],
                                 start=True, stop=True)
                atT = state.tile([128, 97], F32, tag="atT")
                nc.vector.tensor_copy(out=atT[:], in_=pat[:, 0:97])

                # ctx cross-matmul: psum rows 32b hold batch-b context
                pctx = ps_sc.tile([97, BL * H], F32, tag="sc")
                for half in range(2):
                    nc.tensor.matmul(
                        out=pctx[:, half * 512:(half + 1) * 512],
                        lhsT=atT[:],
                        rhs=encR_sb[:, half * 512:(half + 1) * 512],
                        start=True, stop=True,
                    )
                for b in range(BL):
                    if b % 2 == 0:
                        nc.vector.tensor_scalar_mul(
                            ctx_spread[32 * b:32 * b + 1, :],
                            pctx[32 * b:32 * b + 1, b * H:(b + 1) * H],
                            rcp[32 * b:32 * b + 1, 0:1],
                        )
                    else:
                        nc.scalar.mul(
                            ctx_spread[32 * b:32 * b + 1, :],
                            pctx[32 * b:32 * b + 1, b * H:(b + 1) * H],
                            rcp[32 * b:32 * b + 1, 0:1],
                        )
                # ctx_spread -> packed predT ctx-half via gatherp
                pct = ps_t.tile([128, 2 * 128], F32, tag="pt")
                for k in range(2):
                    nc.tensor.matmul(
                        out=pct[:, 128 * k:128 * k + BL],
                        lhsT=ctx_spread[:, k * 128:(k + 1) * 128],
                        rhs=gatherp[:],
                        start=True, stop=True,
                    )
                for k in range(2):
                    eng = nc.vector.tensor_copy if k == 0 else nc.scalar.copy
                    eng(out=predT[k][:, BL * t:BL * (t + 1)],
                        in_=pct[:, 128 * k:128 * k + BL])

                # ------------- chunk boundary: AllGather + classifier -------------
                if (t + 1) % CHUNK_T == 0 or t == t_steps - 1:
                    c = t // CHUNK_T
                    clen = min(CHUNK_T, t_steps - c * CHUNK_T)
                    cw = BL * clen
                    stg = dram.tile([4, 128, cw], F32R, tag="stage")
                    for k in range(4):
                        # gpsimd: fp32 -> fp32r "cast" (bit-identical)
                        nc.gpsimd.dma_start(
                            out=stg[k], in_=predT[k][:, BL * CHUNK_T * c: BL * CHUNK_T * c + cw]
                        )
                    gout = dram.tile([NCORES, 4, 128, cw], F32R, tag="gather",
                                     addr_space="Shared")
                    nc.gpsimd.collective_compute(
                        "AllGather",
                        Alu.bypass,
                        replica_groups=[list(range(NCORES))],
                        ins=[stg[:].opt()],
                        outs=[gout[:].opt()],
                    )
                    gp = [gath.tile([128, NCORES * cw], F32R, tag=f"gp{k}", name=f"gp{k}") for k in range(4)]
                    for k in range(4):
                        ap = gout[:, k, :, :].rearrange("c p x -> p c x")
                        nc.sync.dma_start(out=gp[k][:], in_=ap)
                    tot = NCORES * cw
                    for mt in range((tot + 127) // 128):
                        n_m_cols = min(128, tot - mt * 128)
                        for n in range(VS // NVC):
                            cls_queue.append(
                                (lambda gpt=gp, cc=c, mm=mt, nm=n_m_cols, nn=n:
                                 emit_cls_unit(gpt, cc, mm, nm, nn))
                            )
                # drain classifier work gradually: ~2 units per step keeps PE fed
                if t >= CHUNK_T:
                    flush_cls(2)

            flush_cls(len(cls_queue))

    nc.compile()
    return nc


_built = {}


def _get_nc(t_steps=T):
    if t_steps not in _built:
        _built[t_steps] = build_nc(t_steps)
    return _built[t_steps]


def make_in_maps(inputs, t_steps=T):
    enc = np.ascontiguousarray(np.asarray(inputs["encoder_state"]), np.float32)
    ihs = np.asarray(inputs["initial_hidden_state"], np.float32)
    tgt = np.asarray(inputs["target_sequence"]).astype(np.int32)[:, :t_steps]
    emb = np.ascontiguousarray(np.asarray(inputs["emb"]), np.float32)
    W_hm = np.asarray(inputs["W_hm"], np.float32)
    b_hm = np.asarray(inputs["b_hm"], np.float32)
    W_ih = np.asarray(inputs["W_ih"], np.float32)
    b_ih = np.asarray(inputs["b_ih"], np.float32)
    W_hh = np.asarray(inputs["W_hh"], np.float32)
    b_hh = np.asarray(inputs["b_hh"], np.float32)
    W_cls = np.asarray(inputs["W_cls"], np.float32)
    b_cls = np.asarray(inputs["b_cls"], np.float32)

    wicT = np.ascontiguousarray(W_ih[:, E:E + H].T.reshape(2, 128, 3 * H))
    wixT = np.ascontiguousarray(W_ih[:, 0:E].T.reshape(2, 128, 3 * H))
    # n-slice (outputs 512:768) pre-halved for the tanh-based sigmoid folding
    W_hh_s = W_hh.copy()
    W_hh_s[512:768, :] *= 0.5
    whhT = np.ascontiguousarray(W_hh_s.T.reshape(2, 128, 3 * H))
    whmT = np.ascontiguousarray(W_hm.T.reshape(2, 128, H))
    bgx = np.concatenate([(b_ih + b_hh)[0:512], b_ih[512:768]])[None, :]
    bgx = np.ascontiguousarray(bgx, np.float32)
    bhhn = np.ascontiguousarray(0.5 * b_hh[None, 512:768])
    bhm = np.ascontiguousarray(b_hm[None, :])

    spreadp = np.zeros((BL, 97), np.float32)
    s97 = np.zeros((97, 97), np.float32)
    gatherp = np.zeros((97, BL), np.float32)
    for b in range(BL):
        spreadp[b, 32 * b] = 1.0
        s97[32 * b, 32 * b] = 1.0
        gatherp[32 * b, b] = 1.0

    in_maps = []
    for i in range(NCORES):
        bs = slice(BL * i, BL * (i + 1))
        e = enc[bs]                                   # [4, 128, 256]
        encT = np.ascontiguousarray(
            e.transpose(2, 0, 1).reshape(2, 128, BL * SRC))
        encR = np.ascontiguousarray(
            e.transpose(1, 0, 2).reshape(SRC, BL * H))
        ihsT = np.ascontiguousarray(ihs[bs].T.reshape(2, 128, BL))
        tokf = np.ascontiguousarray(tgt[bs].T.reshape(-1))   # t-major: 4t+b
        n_tok = BL * t_steps
        pad = (-n_tok) % 128
        tokp = np.concatenate([tokf, np.zeros(pad, np.int32)]).astype(np.int32)
        tok = np.ascontiguousarray(tokp.reshape(-1, 128, 1))
        vs = slice(VS * i, VS * (i + 1))
        wclsT = np.ascontiguousarray(W_cls[vs].T.reshape(4, 128, VS))
        bcls = np.ascontiguousarray(b_cls[None, vs])
        in_maps.append({
            "encT": encT, "encR": encR, "wicT": wicT, "whhT": whhT,
            "wixT": wixT, "whmT": whmT, "ihsT": ihsT, "bgx": bgx,
            "bhhn": bhhn, "bhm": bhm, "bcls": bcls, "emb": emb,
            "tok": tok, "wclsT": wclsT,
            "spreadp": spreadp, "s97": s97, "gatherp": gatherp,
        })
    return in_maps


def unshard_logits(per_core, t_steps=T):
    """per_core: list of [NCORES*BL*t_steps, VS] arrays (vocab shards)."""
    parts = []
    n_chunks = (t_steps + CHUNK_T - 1) // CHUNK_T
    for out2 in per_core:
        blocks = []
        for c in range(n_chunks):
            clen = min(CHUNK_T, t_steps - c * CHUNK_T)
            r0 = NCORES * BL * CHUNK_T * c
            blk = out2[r0:r0 + NCORES * BL * clen].reshape(NCORES, clen, BL, VS)
            blocks.append(blk)
        arr = np.concatenate(blocks, axis=1)          # [cores, T, BL, VS]
        parts.append(arr.transpose(0, 2, 1, 3).reshape(B, t_steps, VS))
    return np.concatenate(parts, axis=2)


def kernel(**inputs) -> np.ndarray:
    nc = _get_nc(T)
    in_maps = make_in_maps(inputs, T)
    res = run_bass_kernel_spmd(nc, in_maps, list(range(NCORES)))
    return unshard_logits([res.results[i]["logits"] for i in range(NCORES)], T)


# revision 28
# speedup vs baseline: 1.0940x; 1.0940x over previous
"""Trainium2 Bass kernel for a GRU decoder with attention + vocab classifier.

Model (per reference):
  h0 = ihs @ W_hm.T + b_hm ; ctx0 = 0
  per step t (teacher forcing):
    gi = [x_t, ctx] @ W_ih.T + b_ih ; gh = h @ W_hh.T + b_hh
    r = sig(gi_r + gh_r); z = sig(gi_z + gh_z); n = tanh(gi_n + r * gh_n)
    h' = (1-z)*n + z*h
    scores = einsum('bsh,bh->bs', enc, h'); attn = softmax(scores)
    ctx' = einsum('bsh,bs->bh', enc, attn)
    logits_t = [ctx', h'] @ W_cls.T + b_cls

Distribution (8 cores):
  - recurrence batch-sharded: core i owns batches [4i, 4i+4); exact fp32
  - pred = [ctx', h'] states AllGathered in chunks of 8 timesteps
  - classifier vocab-sharded (fp32r matmuls): core i owns vocab rows
    [4000i, 4000(i+1)), computes its slice for ALL batches; host concats.

Numerics: the recurrence uses exact-fp32 matmuls (fp32r ~ tf32 is too
coarse for this chaotic RNN's feedback path). Sigmoids are computed as
sig(x) = (tanh(x/2)+1)/2 so every ACT function (tanh/exp/copy) lives in
one activation table -> no per-step ACT_TABLE_LOADs. The 1/2 factors are
folded: tanh gets scale=0.5; W_hh/b_hh n-slices are pre-halved on host;
the z-path applies one extra 0.5 multiply.

Batch-partition constraint: compute-engine SBUF accesses must start at
partition 0/32/64/96, so cross-matmul outputs place batch b at partition
32b ("spread" layout) via small selector matrices, and packed forms are
derived with strided-free copies.
"""

import numpy as np

import concourse.bacc as bacc
import concourse.bass as bass
import concourse.mybir as mybir
from concourse import masks, tile
from concourse.bass_utils import run_bass_kernel_spmd

B, SRC, T, H, E, V = 32, 128, 64, 256, 256, 32000
NCORES = 8
BL = B // NCORES          # 4 local batches
VS = V // NCORES          # 4000 vocab shard
CHUNK_T = 8               # timesteps per AllGather/classifier chunk
NVC = 500                 # vocab cols per classifier matmul
GX_AHEAD = 6              # per-step Gx row DMA prefetch distance

dt = mybir.dt
F32, F32R, I32 = dt.float32, dt.float32r, dt.int32
Alu = mybir.AluOpType
Act = mybir.ActivationFunctionType


def build_nc(t_steps=T):
    nc = bacc.Bacc(None, target_bir_lowering=False, debug=True)
    n_tok = BL * t_steps                      # local tokens, t-major: col = 4t+b
    n_mt = (n_tok + 127) // 128               # token tiles for Gx

    # ---------------- DRAM I/O ----------------
    encT_d = nc.dram_tensor("encT", [2, 128, BL * SRC], F32, kind="ExternalInput")
    encR_d = nc.dram_tensor("encR", [SRC, BL * H], F32, kind="ExternalInput")
    wicT_d = nc.dram_tensor("wicT", [2, 128, 3 * H], F32, kind="ExternalInput")
    whhT_d = nc.dram_tensor("whhT", [2, 128, 3 * H], F32, kind="ExternalInput")
    wixT_d = nc.dram_tensor("wixT", [2, 128, 3 * H], F32, kind="ExternalInput")
    whmT_d = nc.dram_tensor("whmT", [2, 128, H], F32, kind="ExternalInput")
    ihsT_d = nc.dram_tensor("ihsT", [2, 128, BL], F32, kind="ExternalInput")
    bgx_d = nc.dram_tensor("bgx", [1, 3 * H], F32, kind="ExternalInput")
    bhhn_d = nc.dram_tensor("bhhn", [1, H], F32, kind="ExternalInput")
    bhm_d = nc.dram_tensor("bhm", [1, H], F32, kind="ExternalInput")
    bcls_d = nc.dram_tensor("bcls", [1, VS], F32, kind="ExternalInput")
    emb_d = nc.dram_tensor("emb", [V, E], F32, kind="ExternalInput")
    tok_d = nc.dram_tensor("tok", [n_mt, 128, 1], I32, kind="ExternalInput")
    wclsT_d = nc.dram_tensor("wclsT", [4, 128, VS], F32R, kind="ExternalInput")
    # spread/gather selector matrices (constants; batch b <-> partition 32b)
    spreadp_d = nc.dram_tensor("spreadp", [BL, 97], F32, kind="ExternalInput")
    s97_d = nc.dram_tensor("s97", [97, 97], F32, kind="ExternalInput")
    gatherp_d = nc.dram_tensor("gatherp", [97, BL], F32, kind="ExternalInput")
    # rows ordered [chunk][core][t_local][b_local]; host permutes to [B, T, VS]
    out_d = nc.dram_tensor("logits", [NCORES * n_tok, VS], F32, kind="ExternalOutput")

    with tile.TileContext(nc) as tc:
        with (
            tc.tile_pool(name="const", bufs=1) as const,
            tc.tile_pool(name="state", bufs=2) as state,
            tc.tile_pool(name="gxs", bufs=GX_AHEAD + 2) as gxsp,
            tc.tile_pool(name="gath", bufs=2) as gath,
            tc.tile_pool(name="clsst", bufs=2) as clsst,
            tc.tile_pool(name="ps_g", bufs=1, space="PSUM") as ps_g,
            tc.tile_pool(name="ps_sc", bufs=1, space="PSUM") as ps_sc,
            tc.tile_pool(name="ps_t", bufs=2, space="PSUM") as ps_t,
            tc.tile_pool(name="ps_cls", bufs=2, space="PSUM") as ps_cls,
            tc.tile_pool(name="dram", bufs=2, space="DRAM") as dram,
        ):
            # ------------- load constants -------------
            ident = const.tile([128, 128], F32)
            masks.make_identity(nc, ident[:])

            encT_sb = [const.tile([128, BL * SRC], F32, tag=f"encT{k}", name=f"encT{k}") for k in range(2)]
            for k in range(2):
                nc.sync.dma_start(out=encT_sb[k][:], in_=encT_d[k])
            encR_sb = const.tile([SRC, BL * H], F32)
            nc.sync.dma_start(out=encR_sb[:], in_=encR_d[:])

            wic_sb = [const.tile([128, 3 * H], F32, tag=f"wic{k}", name=f"wic{k}") for k in range(2)]
            whh_sb = [const.tile([128, 3 * H], F32, tag=f"whh{k}", name=f"whh{k}") for k in range(2)]
            wix_sb = [const.tile([128, 3 * H], F32, tag=f"wix{k}", name=f"wix{k}") for k in range(2)]
            whm_sb = [const.tile([128, H], F32, tag=f"whm{k}", name=f"whm{k}") for k in range(2)]
            ihsT_sb = [const.tile([128, BL], F32, tag=f"ihsT{k}", name=f"ihsT{k}") for k in range(2)]
            for k in range(2):
                nc.sync.dma_start(out=wic_sb[k][:], in_=wicT_d[k])
                nc.sync.dma_start(out=whh_sb[k][:], in_=whhT_d[k])
                nc.sync.dma_start(out=wix_sb[k][:], in_=wixT_d[k])
                nc.sync.dma_start(out=whm_sb[k][:], in_=whmT_d[k])
                nc.sync.dma_start(out=ihsT_sb[k][:], in_=ihsT_d[k])

            wcls_sb = [const.tile([128, VS], F32R, tag=f"wcls{k}", name=f"wcls{k}") for k in range(4)]
            for k in range(4):
                nc.sync.dma_start(out=wcls_sb[k][:], in_=wclsT_d[k])

            bgx_rep = const.tile([128, 3 * H], F32)
            nc.sync.dma_start(out=bgx_rep[:], in_=bgx_d[:].to_broadcast([128, 3 * H]))
            bhhn_rep = const.tile([BL, H], F32)
            nc.sync.dma_start(out=bhhn_rep[:], in_=bhhn_d[:].to_broadcast([BL, H]))
            bhm_rep = const.tile([BL, H], F32)
            nc.sync.dma_start(out=bhm_rep[:], in_=bhm_d[:].to_broadcast([BL, H]))
            bcls_rep = const.tile([128, VS], F32)
            nc.sync.dma_start(out=bcls_rep[:], in_=bcls_d[:].to_broadcast([128, VS]))
            spreadp = const.tile([BL, 97], F32)
            nc.sync.dma_start(out=spreadp[:], in_=spreadp_d[:])
            s97 = const.tile([97, 97], F32)
            nc.sync.dma_start(out=s97[:], in_=s97_d[:])
            gatherp = const.tile([97, BL], F32)
            nc.sync.dma_start(out=gatherp[:], in_=gatherp_d[:])

            # ------------- embedding gather + Gx -------------
            gx_dram = dram.tile([n_mt * 128, 3 * H], F32, bufs=1)
            X = [const.tile([128, E], F32, tag=f"X{m}", name=f"X{m}") for m in range(n_mt)]
            XT = [const.tile([128, n_mt * 128], F32, tag=f"XT{k}", name=f"XT{k}") for k in range(2)]
            for m in range(n_mt):
                idx = const.tile([128, 1], I32, tag=f"idx{m}")
                nc.sync.dma_start(out=idx[:], in_=tok_d[m])
                nc.gpsimd.indirect_dma_start(
                    out=X[m][:],
                    out_offset=None,
                    in_=emb_d[:],
                    in_offset=bass.IndirectOffsetOnAxis(ap=idx[:, :1], axis=0),
                )
                tokf = const.tile([128, 1], F32, tag=f"tokf{m}")
                nc.vector.tensor_copy(out=tokf[:], in_=idx[:])
                nc.vector.tensor_scalar_min(tokf[:], tokf[:], 1.0)
                # zero out padding_idx=0 rows
                nc.vector.tensor_scalar_mul(X[m][:], X[m][:], tokf[:, 0:1])
                # transpose X -> XT
                pxt = ps_sc.tile([128, 768], F32, tag="sc")
                for k in range(2):
                    nc.tensor.transpose(
                        out=pxt[:, k * 128:(k + 1) * 128],
                        in_=X[m][:, k * 128:(k + 1) * 128],
                        identity=ident[:],
                    )
                for k in range(2):
                    nc.vector.tensor_copy(
                        out=XT[k][:, m * 128:(m + 1) * 128],
                        in_=pxt[:, k * 128:(k + 1) * 128],
                    )
            for m in range(n_mt):
                pgx = ps_sc.tile([128, 768], F32, tag="sc")
                for lo, hi in ((0, 512), (512, 768)):
                    for k in range(2):
                        nc.tensor.matmul(
                            out=pgx[:, lo:hi],
                            lhsT=XT[k][:, m * 128:(m + 1) * 128],
                            rhs=wix_sb[k][:, lo:hi],
                            start=(k == 0),
                            stop=(k == 1),
                        )
                gx_sb = state.tile([128, 3 * H], F32, tag="gx_sb")
                nc.vector.tensor_tensor(out=gx_sb[:], in0=pgx[:], in1=bgx_rep[:], op=Alu.add)
                nc.sync.dma_start(out=gx_dram[m * 128:(m + 1) * 128, :], in_=gx_sb[:])

            # per-step Gx row staging (DMA prefetch; arbitrary partitions OK)
            gx_t = {}

            def prefetch_gx(t):
                if t < t_steps and t not in gx_t:
                    g = gxsp.tile([BL, 3 * H], F32, tag="gxt", name=f"gxt{t}")
                    nc.sync.dma_start(out=g[:], in_=gx_dram[BL * t:BL * (t + 1), :])
                    gx_t[t] = g

            for t0 in range(GX_AHEAD):
                prefetch_gx(t0)

            # ------------- h0 -------------
            ph0 = ps_g.tile([BL, 2 * H], F32, tag="rz")
            for k in range(2):
                nc.tensor.matmul(
                    out=ph0[:, 0:H],
                    lhsT=ihsT_sb[k][:],
                    rhs=whm_sb[k][:],
                    start=(k == 0),
                    stop=(k == 1),
                )
            h_prev = state.tile([BL, H], F32, tag="h")
            nc.vector.tensor_tensor(out=h_prev[:], in0=ph0[:, 0:H], in1=bhm_rep[:], op=Alu.add)
            h0T = const.tile([128, 2 * BL], F32)
            pt0 = ps_t.tile([128, 2 * 128], F32, tag="pt")
            for k in range(2):
                nc.tensor.transpose(
                    out=pt0[:, k * BL:(k + 1) * BL],
                    in_=h_prev[:, k * 128:(k + 1) * 128],
                    identity=ident[0:BL, 0:BL],
                )
            nc.vector.tensor_copy(out=h0T[:], in_=pt0[:, 0:2 * BL])

            # predT: [ctxT(2 tiles); hT(2 tiles)], col 4t+b holds step-t output state
            predT = [const.tile([128, n_tok], F32, tag=f"predT{k}", name=f"predT{k}") for k in range(4)]

            # spread-layout scratch: batch b lives at partition 32*b
            sc_spread = const.tile([97, SRC], F32)
            nc.vector.memset(sc_spread[:], 0.0)
            ctx_spread = const.tile([97, H], F32)
            nc.vector.memset(ctx_spread[:], 0.0)

            # classifier work queue, flushed gradually
            cls_queue = []

            def emit_cls_unit(gp_tiles, c, mt, n_m_cols, n):
                pcls = ps_cls.tile([128, 512], F32, tag="cls")
                for k in range(4):
                    nc.tensor.matmul(
                        out=pcls[:n_m_cols, 0:NVC],
                        lhsT=gp_tiles[k][:, mt * 128: mt * 128 + n_m_cols],
                        rhs=wcls_sb[k][:, n * NVC:(n + 1) * NVC],
                        start=(k == 0),
                        stop=(k == 3),
                    )
                st = clsst.tile([128, NVC], F32, tag="clsst")
                nc.vector.tensor_tensor(
                    out=st[:n_m_cols, :],
                    in0=pcls[:n_m_cols, 0:NVC],
                    in1=bcls_rep[:n_m_cols, n * NVC:(n + 1) * NVC],
                    op=Alu.add,
                )
                r0 = NCORES * BL * CHUNK_T * c + mt * 128
                ap = out_d[r0:r0 + n_m_cols, n * NVC:(n + 1) * NVC]
                nc.sync.dma_start(out=ap, in_=st[:n_m_cols, :])

            def flush_cls(k_units):
                for _ in range(k_units):
                    if cls_queue:
                        cls_queue.pop(0)()

            # ------------- recurrence -------------
            for t in range(t_steps):
                prefetch_gx(t + GX_AHEAD)
                gx = gx_t[t]

                def ctxT_ap(k, t=t):
                    return predT[k][:, BL * (t - 1):BL * t]

                def hT_ap(k, t=t):
                    if t == 0:
                        return h0T[:, k * BL:(k + 1) * BL]
                    return predT[2 + k][:, BL * (t - 1):BL * t]

                # gate matmuls
                pz = ps_g.tile([BL, 2 * H], F32, tag="rz")
                inhn = ps_g.tile([BL, 2 * H], F32, tag="inhn")
                pin = inhn[:, 0:H]
                phn = inhn[:, H:2 * H]
                for k in range(2):
                    nc.tensor.matmul(
                        out=pz[:], lhsT=hT_ap(k), rhs=whh_sb[k][:, 0:512],
                        start=(k == 0), stop=(t == 0 and k == 1),
                    )
                for k in range(2):
                    # n-slice of whh is pre-halved on host
                    nc.tensor.matmul(
                        out=phn, lhsT=hT_ap(k), rhs=whh_sb[k][:, 512:768],
                        start=(k == 0), stop=(k == 1),
                    )
                if t > 0:
                    for k in range(2):
                        nc.tensor.matmul(
                            out=pz[:], lhsT=ctxT_ap(k), rhs=wic_sb[k][:, 0:512],
                            start=False, stop=(k == 1),
                        )
                    for k in range(2):
                        nc.tensor.matmul(
                            out=pin, lhsT=ctxT_ap(k), rhs=wic_sb[k][:, 512:768],
                            start=(k == 0), stop=(k == 1),
                        )

                # gate math; sig(x) = (tanh(x/2)+1)/2
                a_rz = state.tile([BL, 2 * H], F32, tag="a_rz")
                nc.vector.tensor_tensor(out=a_rz[:], in0=pz[:], in1=gx[:, 0:512], op=Alu.add)
                u_rz = state.tile([BL, 2 * H], F32, tag="u_rz")
                nc.scalar.activation(u_rz[:], a_rz[:], Act.Tanh, scale=0.5)
                # hnb = 0.5*(gh_n + b_hh_n)   (0.5 pre-folded into whh/bhhn)
                hnb = state.tile([BL, H], F32, tag="hnb")
                nc.vector.tensor_tensor(out=hnb[:], in0=phn, in1=bhhn_rep[:], op=Alu.add)
                # r*gh_n = (u_r+1)*hnb = u_r*hnb + hnb
                m1 = state.tile([BL, H], F32, tag="m1")
                nc.vector.tensor_tensor(out=m1[:], in0=u_rz[:, 0:H], in1=hnb[:], op=Alu.mult)
                s1 = state.tile([BL, H], F32, tag="s1")
                nc.vector.tensor_tensor(out=s1[:], in0=m1[:], in1=hnb[:], op=Alu.add)
                inn = state.tile([BL, H], F32, tag="inn")
                if t > 0:
                    nc.vector.tensor_tensor(out=inn[:], in0=pin, in1=gx[:, 512:768], op=Alu.add)
                    inn_ap = inn[:]
                else:
                    inn_ap = gx[:, 512:768]
                npre = state.tile([BL, H], F32, tag="npre")
                nc.vector.tensor_tensor(out=npre[:], in0=s1[:], in1=inn_ap, op=Alu.add)
                nn = state.tile([BL, H], F32, tag="nn")
                nc.scalar.activation(nn[:], npre[:], Act.Tanh)
                # h' = nn + z*(h-nn), z = (u_z+1)/2  -> h' = nn + 0.5*(u_z*d + d)
                d = state.tile([BL, H], F32, tag="d")
                nc.vector.tensor_tensor(out=d[:], in0=h_prev[:], in1=nn[:], op=Alu.subtract)
                m2 = state.tile([BL, H], F32, tag="m2")
                nc.vector.tensor_tensor(out=m2[:], in0=u_rz[:, H:2 * H], in1=d[:], op=Alu.mult)
                s2 = state.tile([BL, H], F32, tag="s2")
                nc.vector.tensor_tensor(out=s2[:], in0=m2[:], in1=d[:], op=Alu.add)
                s2h = state.tile([BL, H], F32, tag="s2h")
                nc.vector.tensor_scalar_mul(s2h[:], s2[:], 0.5)
                h_new = state.tile([BL, H], F32, tag="h")
                nc.vector.tensor_tensor(out=h_new[:], in0=nn[:], in1=s2h[:], op=Alu.add)
                h_prev = h_new

                # h_new -> spread-transposed [128, 97] (col 32b = batch b)
                # and packed predT h-half, via selector matmuls
                pt = ps_t.tile([128, 2 * 128], F32, tag="pt")
                hts = [state.tile([128, 97], F32, tag=f"hts{k}", name=f"hts{k}")
                       for k in range(2)]
                for k in range(2):
                    nc.tensor.matmul(
                        out=pt[:, 128 * k:128 * k + 97],
                        lhsT=h_new[:, k * 128:(k + 1) * 128],
                        rhs=spreadp[:],
                        start=True, stop=True,
                    )
                for k in range(2):
                    eng = nc.vector.tensor_copy if k == 0 else nc.scalar.copy
                    eng(out=hts[k][:], in_=pt[:, 128 * k:128 * k + 97])
                for k in range(2):
                    eng = nc.scalar.copy if k == 0 else nc.vector.tensor_copy
                    eng(out=predT[2 + k][:, BL * t:BL * (t + 1)],
                        in_=pt[:, 128 * k:128 * (k + 1)]
                        .rearrange("p (b u) -> p b u", u=32)[:, :, 0:1])

                # scores: psum rows 32b hold batch-b scores
                psc = ps_sc.tile([97, BL * H], F32, tag="sc")
                for k in range(2):
                    nc.tensor.matmul(
                        out=psc[:, 0:BL * SRC],
                        lhsT=hts[k][:],
                        rhs=encT_sb[k][:],
                        start=(k == 0), stop=(k == 1),
                    )
                # extract diag rows into spread layout (partition 32*b)
                for b in range(BL):
                    eng = nc.vector.tensor_copy if b % 2 == 0 else nc.scalar.copy
                    eng(out=sc_spread[32 * b:32 * b + 1, :],
                        in_=psc[32 * b:32 * b + 1, b * SRC:(b + 1) * SRC])

                # softmax (unnormalized; 1/sum folded into ctx extraction)
                negmax = state.tile([97, 1], F32, tag="negmax")
                nc.vector.tensor_reduce(
                    out=negmax[:], in_=sc_spread[:], axis=mybir.AxisListType.X,
                    op=Alu.max, negate=True,
                )
                attn = state.tile([97, SRC], F32, tag="attn")
                sume = state.tile([97, 1], F32, tag="sume")
                nc.scalar.activation(
                    attn[:], sc_spread[:], Act.Exp,
                    bias=negmax[:, 0:1], accum_out=sume[:, 0:1],
                )
                rcp = state.tile([97, 1], F32, tag="rcp")
                nc.vector.reciprocal(rcp[:], sume[:])

                # attn [97, 128] -> spread-transposed atT [128, 97] via s97
                pat = ps_t.tile([128, 2 * 128], F32, tag="pt")
                nc.tensor.matmul(out=pat[:, 0:97], lhsT=attn[:], rhs=s97[:],
                                 start=True, stop=True)
                atT = state.tile([128, 97], F32, tag="atT")
                nc.vector.tensor_copy(out=atT[:], in_=pat[:, 0:97])

                # ctx cross-matmul: psum rows 32b hold batch-b context
                pctx = ps_sc.tile([97, BL * H], F32, tag="sc")
                for half in range(2):
                    nc.tensor.matmul(
                        out=pctx[:, half * 512:(half + 1) * 512],
                        lhsT=atT[:],
                        rhs=encR_sb[:, half * 512:(half + 1) * 512],
                        start=True, stop=True,
                    )
                for b in range(BL):
                    if b % 2 == 0:
                        nc.vector.tensor_scalar_mul(
                            ctx_spread[32 * b:32 * b + 1, :],
                            pctx[32 * b:32 * b + 1, b * H:(b + 1) * H],
                            rcp[32 * b:32 * b + 1, 0:1],
                        )
                    else:
                        nc.scalar.mul(
                            ctx_spread[32 * b:32 * b + 1, :],
                            pctx[32 * b:32 * b + 1, b * H:(b + 1) * H],
                            rcp[32 * b:32 * b + 1, 0:1],
                        )
                # ctx_spread -> packed predT ctx-half via gatherp
                pct = ps_t.tile([128, 2 * 128], F32, tag="pt")
                for k in range(2):
                    nc.tensor.matmul(
                        out=pct[:, 128 * k:128 * k + BL],
                        lhsT=ctx_spread[:, k * 128:(k + 1) * 128],
                        rhs=gatherp[:],
                        start=True, stop=True,
                    )
                for k in range(2):
                    eng = nc.vector.tensor_copy if k == 0 else nc.scalar.copy
                    eng(out=predT[k][:, BL * t:BL * (t + 1)],
                        in_=pct[:, 128 * k:128 * k + BL])

                # ------------- chunk boundary: AllGather + classifier -------------
                if (t + 1) % CHUNK_T == 0 or t == t_steps - 1:
                    c = t // CHUNK_T
                    clen = min(CHUNK_T, t_steps - c * CHUNK_T)
                    cw = BL * clen
                    stg = dram.tile([4, 128, cw], F32R, tag="stage")
                    for k in range(4):
                        # gpsimd: fp32 -> fp32r "cast" (bit-identical)
                        nc.gpsimd.dma_start(
                            out=stg[k], in_=predT[k][:, BL * CHUNK_T * c: BL * CHUNK_T * c + cw]
                        )
                    gout = dram.tile([NCORES, 4, 128, cw], F32R, tag="gather",
                                     addr_space="Shared")
                    nc.gpsimd.collective_compute(
                        "AllGather",
                        Alu.bypass,
                        replica_groups=[list(range(NCORES))],
                        ins=[stg[:].opt()],
                        outs=[gout[:].opt()],
                    )
                    gp = [gath.tile([128, NCORES * cw], F32R, tag=f"gp{k}", name=f"gp{k}") for k in range(4)]
                    for k in range(4):
                        ap = gout[:, k, :, :].rearrange("c p x -> p c x")
                        nc.sync.dma_start(out=gp[k][:], in_=ap)
                    tot = NCORES * cw
                    for mt in range((tot + 127) // 128):
                        n_m_cols = min(128, tot - mt * 128)
                        for n in range(VS // NVC):
                            cls_queue.append(
                                (lambda gpt=gp, cc=c, mm=mt, nm=n_m_cols, nn=n:
                                 emit_cls_unit(gpt, cc, mm, nm, nn))
                            )
                # drain classifier work gradually: ~2 units per step keeps PE fed
                if t >= CHUNK_T:
                    flush_cls(2)

            flush_cls(len(cls_queue))

    nc.compile()
    return nc


_built = {}


def _get_nc(t_steps=T):
    if t_steps not in _built:
        _built[t_steps] = build_nc(t_steps)
    return _built[t_steps]


def make_in_maps(inputs, t_steps=T):
    enc = np.ascontiguousarray(np.asarray(inputs["encoder_state"]), np.float32)
    ihs = np.asarray(inputs["initial_hidden_state"], np.float32)
    tgt = np.asarray(inputs["target_sequence"]).astype(np.int32)[:, :t_steps]
    emb = np.ascontiguousarray(np.asarray(inputs["emb"]), np.float32)
    W_hm = np.asarray(inputs["W_hm"], np.float32)
    b_hm = np.asarray(inputs["b_hm"], np.float32)
    W_ih = np.asarray(inputs["W_ih"], np.float32)
    b_ih = np.asarray(inputs["b_ih"], np.float32)
    W_hh = np.asarray(inputs["W_hh"], np.float32)
    b_hh = np.asarray(inputs["b_hh"], np.float32)
    W_cls = np.asarray(inputs["W_cls"], np.float32)
    b_cls = np.asarray(inputs["b_cls"], np.float32)

    wicT = np.ascontiguousarray(W_ih[:, E:E + H].T.reshape(2, 128, 3 * H))
    wixT = np.ascontiguousarray(W_ih[:, 0:E].T.reshape(2, 128, 3 * H))
    # n-slice (outputs 512:768) pre-halved for the tanh-based sigmoid folding
    W_hh_s = W_hh.copy()
    W_hh_s[512:768, :] *= 0.5
    whhT = np.ascontiguousarray(W_hh_s.T.reshape(2, 128, 3 * H))
    whmT = np.ascontiguousarray(W_hm.T.reshape(2, 128, H))
    bgx = np.concatenate([(b_ih + b_hh)[0:512], b_ih[512:768]])[None, :]
    bgx = np.ascontiguousarray(bgx, np.float32)
    bhhn = np.ascontiguousarray(0.5 * b_hh[None, 512:768])
    bhm = np.ascontiguousarray(b_hm[None, :])

    spreadp = np.zeros((BL, 97), np.float32)
    s97 = np.zeros((97, 97), np.float32)
    gatherp = np.zeros((97, BL), np.float32)
    for b in range(BL):
        spreadp[b, 32 * b] = 1.0
        s97[32 * b, 32 * b] = 1.0
        gatherp[32 * b, b] = 1.0

    in_maps = []
    for i in range(NCORES):
        bs = slice(BL * i, BL * (i + 1))
        e = enc[bs]                                   # [4, 128, 256]
        encT = np.ascontiguousarray(
            e.transpose(2, 0, 1).reshape(2, 128, BL * SRC))
        encR = np.ascontiguousarray(
            e.transpose(1, 0, 2).reshape(SRC, BL * H))
        ihsT = np.ascontiguousarray(ihs[bs].T.reshape(2, 128, BL))
        tokf = np.ascontiguousarray(tgt[bs].T.reshape(-1))   # t-major: 4t+b
        n_tok = BL * t_steps
        pad = (-n_tok) % 128
        tokp = np.concatenate([tokf, np.zeros(pad, np.int32)]).astype(np.int32)
        tok = np.ascontiguousarray(tokp.reshape(-1, 128, 1))
        vs = slice(VS * i, VS * (i + 1))
        wclsT = np.ascontiguousarray(W_cls[vs].T.reshape(4, 128, VS))
        bcls = np.ascontiguousarray(b_cls[None, vs])
        in_maps.append({
            "encT": encT, "encR": encR, "wicT": wicT, "whhT": whhT,
            "wixT": wixT, "whmT": whmT, "ihsT": ihsT, "bgx": bgx,
            "bhhn": bhhn, "bhm": bhm, "bcls": bcls, "emb": emb,
            "tok": tok, "wclsT": wclsT,
            "spreadp": spreadp, "s97": s97, "gatherp": gatherp,
        })
    return in_maps


def unshard_logits(per_core, t_steps=T):
    """per_core: list of [NCORES*BL*t_steps, VS] arrays (vocab shards)."""
    parts = []
    n_chunks = (t_steps + CHUNK_T - 1) // CHUNK_T
    for out2 in per_core:
        blocks = []
        for c in range(n_chunks):
            clen = min(CHUNK_T, t_steps - c * CHUNK_T)
            r0 = NCORES * BL * CHUNK_T * c
            blk = out2[r0:r0 + NCORES * BL * clen].reshape(NCORES, clen, BL, VS)
            blocks.append(blk)
        arr = np.concatenate(blocks, axis=1)          # [cores, T, BL, VS]
        parts.append(arr.transpose(0, 2, 1, 3).reshape(B, t_steps, VS))
    return np.concatenate(parts, axis=2)


def kernel(**inputs) -> np.ndarray:
    nc = _get_nc(T)
    in_maps = make_in_maps(inputs, T)
    res = run_bass_kernel_spmd(nc, in_maps, list(range(NCORES)))
    return unshard_logits([res.results[i]["logits"] for i in range(NCORES)], T)
